# revision 4
# baseline (speedup 1.0000x reference)
"""Causal attention kernel for Trainium2 (Bass/Tile), batch-parallel over 8 cores.

Problem: B=8, S=2048, DK=DV=128 fp32 causal attention
  O = softmax(Q @ K^T / sqrt(128) + causal_mask) @ V

Sharding: one batch element per NeuronCore (8 cores, no collectives).

Per-core plan (ACT-exp is the bottleneck engine at 1 col/cycle @1.2GHz, so the
whole schedule is built to keep ScalarE 100% busy on exactly the causal
triangle and nothing else):
  - q blocks of 512 processed in REVERSE (j=3..0) so the final block is the
    small one (4 chunks) and the tail after the last exp is tiny.
  - scores stream through a 6-bank PSUM ring (2 super-slots x 3 banks); full
    k-chunks [k=128, q=512] are grouped 3 per super-slot so one [128,1536]
    exp amortizes the ~185ns ACT access overhead.
  - the 4 diagonal chunks of each block are trimmed to their visible widths
    (512/384/256/128) and packed [d1|d3|d0|d2] into one super-slot (bank-
    boundary safe); one [128,1280] exp covers them exactly. d3 shares d1's
    bank and relies on PSUM pending-zero auto-clear (start=False opener).
  - diagonal 128x128 pieces get a 0/1 triangular mask multiply on DVE after
    exp; everything exp'd is a real causal column (no wasted ACT cols).
  - AV accumulates per 128-row q strip into PSUM [128,129] regions (V plus a
    ones column for the softmax denominator), two strips per bank, the
    second strip opened via the pending-zero trick; strips finalize on DVE
    (reciprocal + scale) and stream out per-strip [128,128] f32 stores.
  - startup: smallest-possible first DMAs (KT chunk0 / QT block3 on separate
    queues) and a 1-chunk first super-slot so ACT starts ~3us in; the exp
    table load hides behind the DMA shadow via a warm activation.

kernel() verifies the mask really is causal-shaped (zeros on/below the
diagonal, <= -1e4 above); any other mask falls back to an exact host path.
"""

import math
import sys

if "/opt/trn_rl_repo" not in sys.path:
    sys.path.insert(0, "/opt/trn_rl_repo")

import numpy as np
import ml_dtypes

import concourse.bacc as bacc
import concourse.mybir as mybir
import concourse.tile as tile
from concourse.bass_utils import run_bass_kernel_spmd

B, S, DK, DV = 8, 2048, 128, 128
N_CORES = 8
SCALE = 1.0 / math.sqrt(DK)

F32 = mybir.dt.float32
BF16 = mybir.dt.bfloat16

QBLK = 512          # q block width
KCH = 128           # k chunk (partition dim of S^T tiles)
NKC = S // KCH      # 16 k chunks
VW = DV + 1         # V chunk + ones column

# diagonal-quad packing: region col offsets/widths inside a [128,1280] slot,
# laid out [d1|d3|d0|d2] so no matmul output crosses a PSUM bank boundary.
QOFF = {1: 0, 3: 384, 0: 512, 2: 1024}
QWID = {1: 384, 3: 128, 0: 512, 2: 256}
# region emission order and, per strip, the region carrying its stop matmul
QORDER = (1, 3, 0, 2)
LAST_D = {0: 0, 1: 0, 2: 2, 3: 2}

_CACHE = {}


def _build():
    nc = bacc.Bacc(
        "TRN2",
        target_bir_lowering=False,
        debug=False,
        enable_asserts=True,
        num_devices=N_CORES,
    )

    qt_d = nc.dram_tensor("QT", [128, S], BF16, kind="ExternalInput").ap()
    kt_d = nc.dram_tensor("KT", [128, S], BF16, kind="ExternalInput").ap()
    vp_d = nc.dram_tensor("VP", [128, NKC * VW], BF16, kind="ExternalInput").ap()
    bm_d = nc.dram_tensor("BM", [128, 128], BF16, kind="ExternalInput").ap()
    o_d = nc.dram_tensor("O", [S, DV], F32, kind="ExternalOutput").ap()

    Exp = mybir.ActivationFunctionType.Exp

    with tile.TileContext(nc) as tc:
        with (
            tc.tile_pool(name="persist", bufs=1) as persist,
            tc.tile_pool(name="es_pool", bufs=7) as es_pool,
            tc.tile_pool(name="ob_pool", bufs=6) as ob_pool,
            tc.tile_pool(name="rc_pool", bufs=6) as rc_pool,
            tc.tile_pool(name="ps_pool", bufs=2, space="PSUM") as ps_pool,
            tc.tile_pool(name="po_pool", bufs=2, space="PSUM") as po_pool,
        ):
            qt = persist.tile([128, S], BF16, name="qt")
            kt = persist.tile([128, S], BF16, name="kt")
            vp = persist.tile([128, NKC * VW], BF16, name="vp")
            bm = persist.tile([128, 128], BF16, name="bm")

            # ---- input DMAs, ordered by first use (blocks run j=3..0) ----
            # queues: sync=SP HWDGE, scalar=ACT HWDGE (startup only, keeps
            # ACT.SEQ clean once exps stream), gpsimd=SWDGE.
            nc.scalar.dma_start(kt[:, 0:128], kt_d[:, 0:128])
            nc.sync.dma_start(qt[:, 1536:2048], qt_d[:, 1536:2048])
            nc.scalar.dma_start(vp[:, 0 : 4 * VW], vp_d[:, 0 : 4 * VW])
            nc.sync.dma_start(kt[:, 128:1024], kt_d[:, 128:1024])
            nc.sync.dma_start(kt[:, 1024:2048], kt_d[:, 1024:2048])
            nc.gpsimd.dma_start(bm[:], bm_d)
            nc.gpsimd.dma_start(vp[:, 4 * VW : 10 * VW], vp_d[:, 4 * VW : 10 * VW])
            nc.gpsimd.dma_start(qt[:, 1024:1536], qt_d[:, 1024:1536])
            nc.gpsimd.dma_start(vp[:, 10 * VW : 16 * VW], vp_d[:, 10 * VW : 16 * VW])
            nc.gpsimd.dma_start(qt[:, 512:1024], qt_d[:, 512:1024])
            nc.gpsimd.dma_start(qt[:, 0:512], qt_d[:, 0:512])

            # warm activation: forces the Exp table load into the DMA shadow
            warm = persist.tile([128, 1], F32, name="warm")
            nc.vector.memset(warm[:], 0.0)
            nc.scalar.activation(warm[:], warm[:], Exp)

            # ---- chunk stream: reversed blocks, full chunks in groups ----
            slots = []
            for j in (3, 2, 1, 0):
                full = list(range(4 * j))
                if j == 3:
                    groups = [[0]] + [full[i : i + 3] for i in range(1, 12, 3)]
                else:
                    groups = [full[i : i + 3] for i in range(0, len(full), 3)]
                for g in groups:
                    if g:
                        slots.append(("full", j, g))
                slots.append(("quad", j, None))

            po_tiles = {}   # j -> {qs: (tile, col)}
            opened = {}     # id(tile) -> bool
            started = {}    # (j, qs) -> bool

            def emit_S(slot, sid):
                kind, j, g = slot
                ps = ps_pool.tile([128, 1536], F32, name=f"ps_{sid}", tag="ps")
                es = es_pool.tile([128, 1536], BF16, name=f"es_{sid}", tag="es")
                if kind == "full":
                    for t, c in enumerate(g):
                        nc.tensor.matmul(
                            ps[:, 512 * t : 512 * (t + 1)],
                            kt[:, 128 * c : 128 * (c + 1)],
                            qt[:, 512 * j : 512 * (j + 1)],
                            start=True,
                            stop=True,
                        )
                    w = 512 * len(g)
                    nc.scalar.activation(es[:, 0:w], ps[:, 0:w], Exp, scale=SCALE)
                else:
                    for d in QORDER:
                        c = 4 * j + d
                        nc.tensor.matmul(
                            ps[:, QOFF[d] : QOFF[d] + QWID[d]],
                            kt[:, 128 * c : 128 * (c + 1)],
                            qt[:, 512 * j + 128 * d : 512 * (j + 1)],
                            start=(d != 3),
                            stop=True,
                            skip_group_check=(d == 3),
                        )
                    if j == 0:
                        # split so d1/d3 AVs (and the whole pipeline drain)
                        # overlap the final exp
                        nc.scalar.activation(es[:, 0:512], ps[:, 0:512], Exp, scale=SCALE)
                        nc.scalar.activation(es[:, 512:1280], ps[:, 512:1280], Exp, scale=SCALE)
                    else:
                        nc.scalar.activation(es[:, 0:1280], ps[:, 0:1280], Exp, scale=SCALE)
                    # 0/1 triangular mask on each diagonal 128x128 piece
                    for d in QORDER:
                        cs = slice(QOFF[d], QOFF[d] + 128)
                        nc.vector.tensor_mul(es[:, cs], es[:, cs], bm[:])
                return es

            def finalize(j, qs, tileq, col):
                rc = rc_pool.tile([128, 1], F32, name=f"rc_{j}_{qs}", tag="rc")
                nc.vector.reciprocal(rc[:], tileq[:, col + 128 : col + 129])
                ob = ob_pool.tile([128, 128], F32, name=f"ob_{j}_{qs}", tag="ob")
                nc.vector.tensor_scalar_mul(ob[:], tileq[:, col : col + 128], rc[:])
                q0 = 512 * j + 128 * qs
                eng = nc.sync if qs >= 2 else nc.gpsimd
                eng.dma_start(o_d[q0 : q0 + 128, :], ob[:])

            def emit_AV(slot, es):
                kind, j, g = slot
                if j not in po_tiles:
                    pa = po_pool.tile([128, 512], F32, name=f"poA_{j}", tag="po")
                    pb = po_pool.tile([128, 512], F32, name=f"poB_{j}", tag="po")
                    po_tiles[j] = {0: (pa, 0), 1: (pa, 129), 2: (pb, 0), 3: (pb, 129)}
                    opened[id(pa)] = False
                    opened[id(pb)] = False
                if kind == "full":
                    pieces = [(c, 512 * t, 0) for t, c in enumerate(g)]
                else:
                    pieces = [(4 * j + d, QOFF[d], d) for d in QORDER]
                for c, off, dmin in pieces:
                    for qs in range(dmin, 4):
                        tileq, col = po_tiles[j][qs]
                        lo = off + 128 * (qs - dmin)
                        first = not started.get((j, qs), False)
                        opn = opened[id(tileq)]
                        stop = (kind == "quad") and (dmin == LAST_D[qs])
                        nc.tensor.matmul(
                            tileq[:, col : col + VW],
                            es[:, lo : lo + 128],
                            vp[:, VW * c : VW * (c + 1)],
                            start=(first and not opn),
                            stop=stop,
                            skip_group_check=True,
                        )
                        started[(j, qs)] = True
                        opened[id(tileq)] = True
                        if stop:
                            finalize(j, qs, tileq, col)

            prev = None
            for sid, slot in enumerate(slots):
                es_cur = emit_S(slot, sid)
                if prev is not None:
                    emit_AV(*prev)
                prev = (slot, es_cur)
            emit_AV(*prev)

    nc.compile()
    return nc


def _make_in_maps(Q, K, V):
    # VP[p, c*129+v] = V[c*128+p, v], ones at v=128 (softmax denominator)
    kk = np.arange(128)[:, None]
    qq = np.arange(128)[None, :]
    bm = (qq >= kk).astype(ml_dtypes.bfloat16)
    in_maps = []
    for b in range(Q.shape[0]):
        vrb = V[b].reshape(NKC, 128, DV).transpose(1, 0, 2)
        vpb = np.concatenate([vrb, np.ones((128, NKC, 1), np.float32)], axis=2)
        vpb = np.ascontiguousarray(vpb.reshape(128, NKC * VW)).astype(ml_dtypes.bfloat16)
        in_maps.append(
            {
                "QT": np.ascontiguousarray(Q[b].T).astype(ml_dtypes.bfloat16),
                "KT": np.ascontiguousarray(K[b].T).astype(ml_dtypes.bfloat16),
                "VP": vpb,
                "BM": bm,
            }
        )
    return in_maps


def _mask_is_causal(mask):
    """True if the mask behaves exactly like the standard causal mask: 0 on
    and below the diagonal, very negative (exp underflows to 0) above."""
    m = np.asarray(mask, dtype=np.float32)
    if m.shape != (1, S, S):
        return False
    m = m[0]
    tril = np.tril_indices(S)
    if not np.all(m[tril] == 0.0):
        return False
    triu = np.triu_indices(S, 1)
    return bool(np.all(m[triu] <= -1e4))


def _host_reference(Q, K, V, mask):
    out = np.empty((Q.shape[0], S, DV), dtype=np.float32)
    for b in range(Q.shape[0]):
        s = (Q[b] @ K[b].T) / math.sqrt(DK) + mask[0]
        s -= s.max(axis=-1, keepdims=True)
        e = np.exp(s)
        out[b] = (e / e.sum(axis=-1, keepdims=True)) @ V[b]
    return out


def kernel(Q, K, V, mask):
    Q = np.asarray(Q, dtype=np.float32)
    K = np.asarray(K, dtype=np.float32)
    V = np.asarray(V, dtype=np.float32)
    mask = np.asarray(mask, dtype=np.float32)

    if not _mask_is_causal(mask):
        # unexpected mask: exact (slow) host path
        return _host_reference(Q, K, V, mask)

    if "nc" not in _CACHE:
        _CACHE["nc"] = _build()
    nc = _CACHE["nc"]

    in_maps = _make_in_maps(Q, K, V)
    res = run_bass_kernel_spmd(nc, in_maps, core_ids=list(range(N_CORES)))
    out = np.stack([res.results[b]["O"] for b in range(B)]).astype(np.float32)
    return out
